# revision 1
# baseline (speedup 1.0000x reference)
"""CTLSTM (continuous-time LSTM) Trainium2 kernel.

Strategy (8 NeuronCores, data-parallel over batch):
  - Each core owns 8 of the 64 sequences and runs the full temporal scan.
  - Gate-major layout: gate dim on SBUF partitions (14 tiles of 128),
    batch on the free dim, so all elementwise work is small wide tiles.
  - Input projection xg = x @ Wx.T + (bx+bh) is computed on-device as a
    fp32 matmul into a DRAM scratch, streamed back during the scan.
  - The 8 sequences are split into TWO phase-shifted lanes of 4: while
    lane A runs its elementwise tail, lane B's recurrent matmuls keep
    the PE busy, hiding the cross-engine latency chain.
  - Recurrent matmul per lane-step: 14 gate-tiles x 2 K-chunks of bf16
    stationary Wh tiles against the [128, 4] hidden state.
  - All in-scan activations come from ONE ACT table set (exp_and_others:
    tanh + exp): sigmoid(x) = 0.5 + 0.5*tanh(x/2) (z-gate weights are
    pre-scaled by 2 so z shares the same tanh(x/2) call), and
    softplus(x) = relu(x) + ln1p(exp(-|x|)) with ln1p approximated by a
    degree-5 polynomial (abs err ~2e-6, on GPSIMD) -- no table switches.
  - Outputs are staged gate-major in SBUF, transposed to batch-major via
    the PE every 8 steps, masked, and DMA'd straight to DRAM.
"""

import sys
import numpy as np

B, L_FULL, I, H = 64, 512, 256, 256
NCORES, BC = 8, 8   # cores, sequences per core
NLANE, LB = 2, 4    # lanes per core, sequences per lane
G = 7 * H
NT = 14             # gate tiles of 128

# Tile order (blocks of 128 gate rows): d0,d1, z0,z1, i0,i1, ib0,ib1,
# f0,f1, fb0,fb1, o0,o1.  Original gate offsets in g: i@0, f@256, z@512,
# o@768, d@1024, ib@1280, fb@1536.
PERM_STARTS = [1024, 1152, 512, 640, 0, 128, 1280, 1408, 256, 384,
               1536, 1664, 768, 896]
PERM_ROWS = np.concatenate([np.arange(s, s + 128) for s in PERM_STARTS])
Z_BLOCKS = (2, 3)  # tile indices whose rows get the x2 pre-scale

# ln1p(u) on [0, 1], least-squares fit on a dense grid, degree 5.
_u = np.linspace(0.0, 1.0, 20001)
_c = np.polyfit(_u, np.log1p(_u), 3)[::-1]  # c0..c3
LN1P_C = [float(v) for v in _c] + [0.0, 0.0]

_BUILD_CACHE = {}
DBG_SKIP = set()  # debug: subset of {'xg','flush','chain','mms','pre'}


def _build(L, reps=1):
    """Build + schedule the bass module for sequence length L."""
    sys.path.insert(0, "/opt/trn_rl_repo")
    import concourse.bass as bass
    import concourse.tile as tile
    import concourse.mybir as mybir
    from concourse import bacc
    from contextlib import ExitStack

    f32 = mybir.dt.float32
    bf16 = mybir.dt.bfloat16
    AF = mybir.ActivationFunctionType
    OP = mybir.AluOpType

    BCL = BC * L
    NBLK = L // 8          # 8-step staging blocks
    TC = min(32, L)        # xg stream chunk (steps)
    NCHUNK = L // TC

    nc = bacc.Bacc("TRN2", target_bir_lowering=False, debug=False,
                   num_devices=NCORES)

    x_in = nc.dram_tensor("x", [BCL, I], f32, kind="ExternalInput")
    whT_in = nc.dram_tensor("whT", [128, 28 * 128], bf16, kind="ExternalInput")
    wxT_in = nc.dram_tensor("wxT", [128, 28 * 128], f32, kind="ExternalInput")
    bias_in = nc.dram_tensor("biasg", [128, NT], f32, kind="ExternalInput")
    dtb_in = nc.dram_tensor("dtb", [128, L * 16], f32, kind="ExternalInput")
    mb_in = nc.dram_tensor("mb", [128, L * 16], f32, kind="ExternalInput")
    mcol_in = nc.dram_tensor("mcolT", [128, 2 * NBLK], f32,
                             kind="ExternalInput")
    id_in = nc.dram_tensor("ident", [128, 128], f32, kind="ExternalInput")
    outs = [nc.dram_tensor(f"out{i}", [BC, L + 1, H], f32,
                           kind="ExternalOutput") for i in range(6)]
    xg_dram = nc.dram_tensor("xg_scratch", [NT, BC, 128, L], f32)

    c0, c1, c2, c3, c4, c5 = LN1P_C

    with tile.TileContext(nc) as tc, ExitStack() as ctx:
        const_pool = ctx.enter_context(tc.tile_pool(name="const", bufs=1))
        whT = const_pool.tile([128, 28 * 128], bf16)
        nc.sync.dma_start(whT[:], whT_in[:])
        dtb = const_pool.tile([128, L * 16], f32)
        nc.sync.dma_start(dtb[:], dtb_in[:])
        mb = const_pool.tile([128, L * 16], f32)
        nc.sync.dma_start(mb[:], mb_in[:])
        mcol = const_pool.tile([128, 2 * NBLK], f32)
        nc.sync.dma_start(mcol[:], mcol_in[:])
        ident = const_pool.tile([128, 128], f32)
        nc.sync.dma_start(ident[:], id_in[:])
        biasg = const_pool.tile([128, NT], f32)
        nc.sync.dma_start(biasg[:], bias_in[:])

        # zero out t=0 of every output (don't rely on pre-zeroed buffers)
        zt0 = const_pool.tile([128, 128], f32)
        nc.vector.memset(zt0[:], 0.0)
        zcol = const_pool.tile([128, 8], f32)
        nc.vector.memset(zcol[:], 0.0)
        halfb = const_pool.tile([128, 1], f32)
        nc.vector.memset(halfb[:], 0.5)
        zwide = const_pool.tile([128, 48], f32)
        nc.vector.memset(zwide[:], 0.0)
        for oi in range(6):
            for cc_ in range(2):
                nc.sync.dma_start(outs[oi][:, 0, cc_ * 128:(cc_ + 1) * 128],
                                  zt0[0:8, :])

        for _rep in range(reps):
            # ---------- Phase 1: transpose x to [i, (b,t)] fp32 ----------
            with tc.tile_pool(name="pre_sb", bufs=3) as pre_sb, \
                 tc.tile_pool(name="pre_ps", bufs=2, space="PSUM") as pre_ps, \
                 tc.tile_pool(name="xT_pool", bufs=1) as xT_pool, \
                 tc.tile_pool(name="wx_pool", bufs=1) as wx_pool, \
                 tc.tile_pool(name="mm_ps", bufs=2, space="PSUM") as mm_ps, \
                 tc.tile_pool(name="xg_sb_pool", bufs=3) as xg_sb_pool:
                wxT = wx_pool.tile([128, 28 * 128], f32)
                nc.sync.dma_start(wxT[:], wxT_in[:])
                xT = xT_pool.tile([128, 2 * BCL], f32)
                for blk in range(0 if 'pre' in DBG_SKIP else BCL // 128):
                    xrow = pre_sb.tile([128, I], f32, tag="xrow")
                    nc.sync.dma_start(xrow[:], x_in[blk * 128:(blk + 1) * 128, :])
                    for k in range(2):
                        pst = pre_ps.tile([128, 128], f32, tag="pst")
                        nc.tensor.transpose(pst[:], xrow[:, k * 128:(k + 1) * 128],
                                            ident[:])
                        nc.scalar.activation(
                            xT[:, k * BCL + blk * 128: k * BCL + (blk + 1) * 128],
                            pst[:], AF.Copy)

                # ---------- Phase 2: xg = x @ Wx_p.T + bias (fp32) ----------
                for j in range(0 if 'pre' in DBG_SKIP else NT):
                    for b in range(BC):
                        ps = mm_ps.tile([128, L], f32, tag="ps")
                        nc.tensor.matmul(ps[:], wxT[:, (2 * j) * 128:(2 * j + 1) * 128],
                                         xT[:, 0 * BCL + b * L: 0 * BCL + (b + 1) * L],
                                         start=True, stop=False)
                        nc.tensor.matmul(ps[:], wxT[:, (2 * j + 1) * 128:(2 * j + 2) * 128],
                                         xT[:, 1 * BCL + b * L: 1 * BCL + (b + 1) * L],
                                         start=False, stop=True)
                        xg_sb = xg_sb_pool.tile([128, L], f32, tag="xg_sb")
                        nc.scalar.activation(xg_sb[:], ps[:], AF.Identity,
                                             bias=biasg[:, j:j + 1])
                        nc.sync.dma_start(xg_dram[j, b], xg_sb[:])

            # ---------- Phase 3: the scan (two phase-shifted lanes) ----------
            # Explicit 2-stage software pipeline: per half-step we emit lane X's
            # recurrent matmuls, then the *previous* half-step's elementwise
            # chain (of the other lane), so the PE stays busy while DVE/ACT run.
            with tc.tile_pool(name="xg_buf", bufs=2) as xg_buf_pool, \
                 tc.tile_pool(name="state", bufs=3) as state_pool, \
                 tc.tile_pool(name="gps_d", bufs=3, space="PSUM") as gps_d_pool, \
                 tc.tile_pool(name="gps_zs", bufs=3, space="PSUM") as gps_zs_pool, \
                 tc.tile_pool(name="tp", bufs=2, space="PSUM") as tp_pool, \
                 tc.tile_pool(name="work", bufs=3) as work_pool, \
                 tc.tile_pool(name="stg", bufs=2) as stg_pool, \
                 tc.tile_pool(name="omask", bufs=3) as omask_pool:

                hn_bf = [None] * NLANE
                cn_half = [None] * NLANE
                for ln in range(NLANE):
                    hn_bf[ln] = state_pool.tile([128, 8], bf16, tag=f"hn_bf{ln}",
                                                name=f"hn_bf{ln}")
                    nc.vector.memset(hn_bf[ln][:], 0.0)
                    cn_half[ln] = state_pool.tile([128, 8], f32, tag=f"cn_half{ln}",
                                                  name=f"cn_half{ln}")
                    nc.vector.memset(cn_half[ln][:], 0.0)

                xg_chunks = [None] * NCHUNK

                def load_chunk(ci):
                    t0 = ci * TC
                    buf = xg_buf_pool.tile([128, 112 * TC], f32, tag="xgc",
                                           name=f"xgc{ci}")
                    dst = buf[:].rearrange("p (j b t) -> p j b t", j=NT, b=BC)
                    src = xg_dram[:, :, :, t0:t0 + TC].rearrange("j b p t -> p j b t")
                    nc.sync.dma_start(dst, src)
                    xg_chunks[ci] = buf

                if 'xg' not in DBG_SKIP:
                    load_chunk(0)

                stg = {}

                def emit_mms(ln, t):
                    g_all = gps_d_pool.tile([128, 56], f32, tag="g_all",
                                            name=f"g_all{ln}")
                    hb = hn_bf[ln]
                    for j in range(0 if 'mms' in DBG_SKIP else NT):
                        dst = g_all[:, j * 4:(j + 1) * 4]
                        for k in range(2):
                            nc.tensor.matmul(
                                dst,
                                whT[:, (2 * j + k) * 128:(2 * j + k + 1) * 128],
                                hb[:, k * LB:(k + 1) * LB],
                                start=(k == 0), stop=(k == 1))
                    return g_all, None

                def make_chain(ln, t, g_all, _unused):
                    ci, tau = t // TC, t % TC
                    kappa, blk = t % 8, t // 8
                    tsl = slice(t * 16 + ln * 8, t * 16 + ln * 8 + 8)
                    bsl = slice(ln * LB, (ln + 1) * LB)

                    def chain():
                        if kappa == 0:
                            for nm in ("h", "c", "cb", "o", "d"):
                                stg[(nm, ln)] = stg_pool.tile(
                                    [128, 64], f32, tag=f"stg_{nm}{ln}",
                                    name=f"stg_{nm}{ln}")
                        sl = slice(kappa * 8, kappa * 8 + 8)
                        xgv = xg_chunks[ci][:].rearrange("p (j b t) -> p j b t",
                                                         j=NT, b=BC)
                        if 'xg' in DBG_SKIP:
                            xg_all = zwide[:, 0:56].rearrange(
                                "p (j b) -> p j b", j=14)
                        else:
                            xg_all = xgv[:, :, bsl, tau]

                        gfull = work_pool.tile([128, 56], f32, tag=f"gf{ln}",
                                               name=f"gf{ln}")
                        nc.vector.tensor_tensor(
                            gfull[:].rearrange("p (j b) -> p j b", j=14),
                            g_all[:].rearrange("p (j b) -> p j b", j=14),
                            xg_all, op=OP.add)
                        gd = gfull[:, 0:8]

                        # --- d path: d = relu(gd) + ln1p(exp(-|gd|)) ---
                        ga = work_pool.tile([128, 8], f32, tag=f"ga{ln}",
                                            name=f"ga{ln}")
                        nc.vector.scalar_tensor_tensor(ga[:], gd, -1.0, gd,
                                                       op0=OP.mult, op1=OP.max)
                        uu = work_pool.tile([128, 8], f32, tag=f"uu{ln}",
                                            name=f"uu{ln}")
                        nc.scalar.activation(uu[:], ga[:], AF.Exp, scale=-1.0)
                        pa = work_pool.tile([128, 8], f32, tag=f"pa{ln}",
                                            name=f"pa{ln}")
                        nc.vector.tensor_scalar(pa[:], uu[:], c3, None, op0=OP.mult)
                        pb = work_pool.tile([128, 8], f32, tag=f"pb{ln}",
                                            name=f"pb{ln}")
                        nc.vector.scalar_tensor_tensor(pb[:], pa[:], c2, uu[:],
                                                       op0=OP.add, op1=OP.mult)
                        nc.vector.scalar_tensor_tensor(pb[:], pb[:], c1, uu[:],
                                                       op0=OP.add, op1=OP.mult)
                        # d = max(gd, 0) + poly   (c0 ~ 1e-5 dropped)
                        nc.vector.scalar_tensor_tensor(stg[("d", ln)][:, sl],
                                                       gd, 0.0, pb[:],
                                                       op0=OP.max, op1=OP.add)
                        md = work_pool.tile([128, 8], f32, tag=f"md{ln}",
                                            name=f"md{ln}")
                        nc.vector.tensor_tensor(md[:], stg[("d", ln)][:, sl],
                                                dtb[:, tsl], op=OP.mult)
                        et = work_pool.tile([128, 8], f32, tag=f"et{ln}",
                                            name=f"et{ln}")
                        nc.scalar.activation(et[:], md[:], AF.Exp, scale=-1.0)

                        # --- z + sigmoid gates ---
                        gt = work_pool.tile([128, 48], f32, tag=f"gt{ln}",
                                            name=f"gt{ln}")
                        nc.scalar.activation(gt[:], gfull[:, 8:56], AF.Tanh,
                                             scale=0.5)

                        iz_i = work_pool.tile([128, 8], f32, tag=f"iz_i{ln}",
                                              name=f"iz_i{ln}")
                        nc.vector.scalar_tensor_tensor(iz_i[:], gt[:, 8:16], 1.0,
                                                       gt[:, 0:8], op0=OP.add,
                                                       op1=OP.mult)
                        iz_ib = work_pool.tile([128, 8], f32, tag=f"iz_ib{ln}",
                                               name=f"iz_ib{ln}")
                        nc.vector.scalar_tensor_tensor(iz_ib[:], gt[:, 16:24], 1.0,
                                                       gt[:, 0:8], op0=OP.add,
                                                       op1=OP.mult)
                        fc_f = work_pool.tile([128, 8], f32, tag=f"fc_f{ln}",
                                              name=f"fc_f{ln}")
                        nc.vector.scalar_tensor_tensor(fc_f[:], gt[:, 24:32], 1.0,
                                                       cn_half[ln][:], op0=OP.add,
                                                       op1=OP.mult)
                        fc_fb = work_pool.tile([128, 8], f32, tag=f"fc_fb{ln}",
                                               name=f"fc_fb{ln}")
                        nc.vector.scalar_tensor_tensor(fc_fb[:], gt[:, 32:40], 1.0,
                                                       cn_half[ln][:], op0=OP.add,
                                                       op1=OP.mult)
                        nc.vector.scalar_tensor_tensor(stg[("c", ln)][:, sl],
                                                       iz_i[:], 0.5, fc_f[:],
                                                       op0=OP.mult, op1=OP.add)
                        nc.vector.scalar_tensor_tensor(stg[("cb", ln)][:, sl],
                                                       iz_ib[:], 0.5, fc_fb[:],
                                                       op0=OP.mult, op1=OP.add)
                        nc.vector.tensor_scalar(stg[("o", ln)][:, sl], gt[:, 40:48],
                                                1.0, 0.5, op0=OP.add, op1=OP.mult)

                        # --- decay + new state ---
                        dd = work_pool.tile([128, 8], f32, tag=f"dd{ln}",
                                            name=f"dd{ln}")
                        nc.vector.tensor_tensor(dd[:], stg[("c", ln)][:, sl],
                                                stg[("cb", ln)][:, sl],
                                                op=OP.subtract)
                        de = work_pool.tile([128, 8], f32, tag=f"de{ln}",
                                            name=f"de{ln}")
                        nc.vector.tensor_tensor(de[:], dd[:], et[:], op=OP.mult)
                        ctt = work_pool.tile([128, 8], f32, tag=f"ctt{ln}",
                                             name=f"ctt{ln}")
                        nc.vector.tensor_tensor(ctt[:], de[:],
                                                stg[("cb", ln)][:, sl], op=OP.add)
                        tct = work_pool.tile([128, 8], f32, tag=f"tct{ln}",
                                             name=f"tct{ln}")
                        nc.scalar.activation(tct[:], ctt[:], AF.Tanh)
                        ht = work_pool.tile([128, 8], f32, tag=f"ht{ln}",
                                            name=f"ht{ln}")
                        nc.vector.tensor_tensor(ht[:], stg[("o", ln)][:, sl],
                                                tct[:], op=OP.mult)
                        hn_bf[ln] = state_pool.tile([128, 8], bf16,
                                                    tag=f"hn_bf{ln}",
                                                    name=f"hn_bf{ln}")
                        nc.vector.tensor_tensor(hn_bf[ln][:], ht[:], mb[:, tsl],
                                                op=OP.mult)
                        nc.vector.tensor_tensor(stg[("h", ln)][:, sl], ht[:],
                                                mb[:, tsl], op=OP.mult)
                        cn_half[ln] = state_pool.tile([128, 8], f32,
                                                      tag=f"cn_half{ln}",
                                                      name=f"cn_half{ln}")
                        nc.vector.scalar_tensor_tensor(cn_half[ln][:], ctt[:], 0.5,
                                                       mb[:, tsl], op0=OP.mult,
                                                       op1=OP.mult)

                        if kappa == 7 and 'flush' not in DBG_SKIP:
                            emit_flush(ln, blk)
                    return chain

                def emit_flush(ln, blk):
                    mcol_ap = mcol[:, blk * 2 + ln: blk * 2 + ln + 1]

                    def out_view(oi):
                        return outs[oi][ln * LB:(ln + 1) * LB,
                                        blk * 8 + 1: blk * 8 + 9, :] \
                            .rearrange("b t (c h) -> t c b h", c=2)

                    tp_h = tp_pool.tile([128, 128], f32, tag="tp", name="tp_h")
                    nc.tensor.transpose(tp_h[0:64, :], stg[("h", ln)][:], ident[:])
                    hmm = omask_pool.tile([128, 128], f32, tag="hmm", name="hmm")
                    nc.vector.tensor_scalar_mul(hmm[0:64, :], tp_h[0:64, :],
                                                mcol_ap[0:64])
                    nc.sync.dma_start(out_view(0), hmm[0:64, :])

                    tp_c = tp_pool.tile([128, 128], f32, tag="tp", name="tp_c")
                    nc.tensor.transpose(tp_c[0:64, :], stg[("c", ln)][:], ident[:])
                    cm = omask_pool.tile([128, 128], f32, tag="cm", name="cm")
                    nc.vector.tensor_scalar_mul(cm[0:64, :], tp_c[0:64, :],
                                                mcol_ap[0:64])
                    nc.sync.dma_start(out_view(2), cm[0:64, :])

                    tp_cb = tp_pool.tile([128, 128], f32, tag="tp", name="tp_cb")
                    nc.tensor.transpose(tp_cb[0:64, :], stg[("cb", ln)][:],
                                        ident[:])
                    cbm = omask_pool.tile([128, 128], f32, tag="cbm", name="cbm")
                    nc.vector.tensor_scalar_mul(cbm[0:64, :], tp_cb[0:64, :],
                                                mcol_ap[0:64])
                    nc.sync.dma_start(out_view(3), cbm[0:64, :])

                    tp_o = tp_pool.tile([128, 128], f32, tag="tp", name="tp_o")
                    nc.tensor.transpose(tp_o[0:64, :], stg[("o", ln)][:], ident[:])
                    om = omask_pool.tile([128, 128], f32, tag="om", name="om")
                    nc.vector.tensor_scalar_mul(om[0:64, :], tp_o[0:64, :],
                                                mcol_ap[0:64])
                    nc.sync.dma_start(out_view(4), om[0:64, :])

                    tp_d = tp_pool.tile([128, 128], f32, tag="tp", name="tp_d")
                    nc.tensor.transpose(tp_d[0:64, :], stg[("d", ln)][:], ident[:])
                    dm = omask_pool.tile([128, 128], f32, tag="dm", name="dm")
                    nc.vector.tensor_scalar_mul(dm[0:64, :], tp_d[0:64, :],
                                                mcol_ap[0:64])
                    nc.sync.dma_start(out_view(5), dm[0:64, :])

                    # out1 (afters_h) = o_m * tanh(c_m)
                    tcm = omask_pool.tile([128, 128], f32, tag="tcm", name="tcm")
                    nc.scalar.activation(tcm[0:64, :], cm[0:64, :], AF.Tanh)
                    hm2 = omask_pool.tile([128, 128], f32, tag="hm2", name="hm2")
                    nc.vector.tensor_tensor(hm2[0:64, :], om[0:64, :],
                                            tcm[0:64, :], op=OP.mult)
                    nc.sync.dma_start(out_view(1), hm2[0:64, :])

                pending = []
                for t in range(L):
                    ci, tau = t // TC, t % TC
                    if tau == 0 and ci + 1 < NCHUNK and 'xg' not in DBG_SKIP:
                        load_chunk(ci + 1)
                    for ln in range(NLANE):
                        g_d, g_zs = emit_mms(ln, t)
                        if 'chain' not in DBG_SKIP:
                            if pending:
                                pending.pop(0)()
                            pending.append(make_chain(ln, t, g_d, g_zs))
                while pending:
                    pending.pop(0)()

    nc.finalize()
    return nc


def _prep_shared(Wx, bx, Wh, bh):
    Wh_p = Wh[PERM_ROWS].astype(np.float32).copy()
    Wx_p = Wx[PERM_ROWS].astype(np.float32).copy()
    bias_p = (bx + bh)[PERM_ROWS].astype(np.float32).copy()
    for zb in Z_BLOCKS:
        Wh_p[zb * 128:(zb + 1) * 128] *= 2.0
        Wx_p[zb * 128:(zb + 1) * 128] *= 2.0
        bias_p[zb * 128:(zb + 1) * 128] *= 2.0

    import ml_dtypes
    whT = np.zeros((128, 28 * 128), dtype=ml_dtypes.bfloat16)
    wxT = np.zeros((128, 28 * 128), dtype=np.float32)
    for j in range(NT):
        for k in range(2):
            s = (2 * j + k) * 128
            whT[:, s:s + 128] = Wh_p[j * 128:(j + 1) * 128,
                                     k * 128:(k + 1) * 128].T
            wxT[:, s:s + 128] = Wx_p[j * 128:(j + 1) * 128,
                                     k * 128:(k + 1) * 128].T
    biasg = np.zeros((128, NT), dtype=np.float32)
    for j in range(NT):
        biasg[:, j] = bias_p[j * 128:(j + 1) * 128]
    return whT, wxT, biasg


def _prep_core(xc, dtc, slc, L):
    x_rows = np.ascontiguousarray(xc.reshape(BC * L, I).astype(np.float32))
    t_idx = np.arange(L)
    m = (t_idx[None, :] < slc[:, None]).astype(np.float32)  # [BC, L]
    dt2 = dtc[:, :, 0].astype(np.float32)  # [BC, L]
    # [128, L*16]: column t*16 + lane*8 + c*4 + b' -> value for (b, t)
    # where b = lane*4 + b'
    col_dt = np.empty((L, 2, 2, LB), np.float32)
    col_m = np.empty((L, 2, 2, LB), np.float32)
    for ln in range(NLANE):
        for c in range(2):
            col_dt[:, ln, c, :] = dt2[ln * LB:(ln + 1) * LB, :].T
            col_m[:, ln, c, :] = m[ln * LB:(ln + 1) * LB, :].T
    dtb = np.broadcast_to(col_dt.reshape(1, L * 16), (128, L * 16)).copy()
    mbv = np.broadcast_to(col_m.reshape(1, L * 16), (128, L * 16)).copy()
    # mcolT [128, 2*NBLK]: partition p = kappa*8 + c*4 + b', col = blk*2+lane
    NBLK = L // 8
    mcol = np.zeros((128, 2 * NBLK), dtype=np.float32)
    kap = np.arange(8)
    for blk in range(NBLK):
        for ln in range(NLANE):
            v = m[ln * LB:(ln + 1) * LB, blk * 8:blk * 8 + 8]  # [b', kappa]
            col = np.repeat(v.T[:, None, :], 2, axis=1)  # [kappa, c, b']
            mcol[0:64, blk * 2 + ln] = col.reshape(64)
    return x_rows, dtb, mbv, mcol


class _CachedRunner:
    """Build the sharded jitted executable once; reuse across calls so the
    NEFF is loaded on the devices a single time."""

    def __init__(self, nc):
        sys.path.insert(0, "/opt/trn_rl_repo")
        import jax
        import numpy as _np
        from jax.sharding import Mesh, PartitionSpec
        from jax.experimental.shard_map import shard_map
        from concourse import bass2jax, mybir
        from concourse.bass2jax import _bass_exec_p, partition_id_tensor, \
            install_neuronx_cc_hook
        install_neuronx_cc_hook()
        self.jax = jax
        partition_name = (nc.partition_id_tensor.name
                          if nc.partition_id_tensor else None)
        in_names, out_names, out_avals, zero_outs = [], [], [], []
        for alloc in nc.m.functions[0].allocations:
            if not isinstance(alloc, mybir.MemoryLocationSet):
                continue
            name = alloc.memorylocations[0].name
            if alloc.kind == "ExternalInput":
                if name != partition_name:
                    in_names.append(name)
            elif alloc.kind == "ExternalOutput":
                out_names.append(name)
                shape = tuple(alloc.tensor_shape)
                dtype = mybir.dt.np(alloc.dtype)
                out_avals.append(jax.core.ShapedArray(shape, dtype))
                zero_outs.append(_np.zeros(shape, dtype))
        self.n_params = len(in_names)
        self.in_names = list(in_names)
        self.out_names = out_names
        self.out_avals = out_avals
        self.zero_outs = zero_outs
        n_outs = len(out_avals)
        in_names_all = in_names + out_names
        if partition_name is not None:
            in_names_all.append(partition_name)
        donate = tuple(range(self.n_params, self.n_params + n_outs))

        def _body(*args):
            operands = list(args)
            if partition_name is not None:
                operands.append(partition_id_tensor())
            outs = _bass_exec_p.bind(
                *operands, out_avals=tuple(out_avals),
                in_names=tuple(in_names_all), out_names=tuple(out_names),
                lowering_input_output_aliases=(), sim_require_finite=True,
                sim_require_nnan=True, nc=nc)
            return tuple(outs)

        devices = jax.devices()[:NCORES]
        mesh = Mesh(_np.asarray(devices), ("core",))
        in_specs = (PartitionSpec("core"),) * (self.n_params + n_outs)
        out_specs = (PartitionSpec("core"),) * n_outs
        self.sharded = jax.jit(
            shard_map(_body, mesh=mesh, in_specs=in_specs,
                      out_specs=out_specs, check_rep=False),
            donate_argnums=donate, keep_unused=True)

    def __call__(self, in_maps):
        import numpy as _np
        per_core = [[_np.asarray(m[name]) for name in self.in_names]
                    for m in in_maps]
        concat_in = [
            _np.concatenate([per_core[c][i] for c in range(NCORES)], axis=0)
            for i in range(self.n_params)]
        concat_zeros = [
            _np.zeros((NCORES * z.shape[0], *z.shape[1:]), z.dtype)
            for z in self.zero_outs]
        out_arrs = self.sharded(*concat_in, *concat_zeros)
        return [
            {name: _np.asarray(out_arrs[i]).reshape(
                NCORES, *self.out_avals[i].shape)[c]
             for i, name in enumerate(self.out_names)}
            for c in range(NCORES)]


class _Res:
    def __init__(self, results):
        self.results = results


_RUNNER_CACHE = {}


def _run(nc, in_maps):
    key = id(nc)
    if key not in _RUNNER_CACHE:
        _RUNNER_CACHE[key] = _CachedRunner(nc)
    return _Res(_RUNNER_CACHE[key](in_maps))


def kernel(x, delta_t, seq_lens, Wx, bx, Wh, bh, _L=None):
    L = _L if _L is not None else x.shape[1]
    if L not in _BUILD_CACHE:
        _BUILD_CACHE[L] = _build(L)
    nc = _BUILD_CACHE[L]

    whT, wxT, biasg = _prep_shared(np.asarray(Wx), np.asarray(bx),
                                   np.asarray(Wh), np.asarray(bh))
    ident = np.eye(128, dtype=np.float32)
    x = np.asarray(x)
    delta_t = np.asarray(delta_t)
    seq_lens = np.asarray(seq_lens)

    in_maps = []
    for k in range(NCORES):
        sl = slice(k * BC, (k + 1) * BC)
        x_rows, dtb, mbv, mcol = _prep_core(x[sl], delta_t[sl], seq_lens[sl], L)
        in_maps.append({
            "x": x_rows, "whT": whT, "wxT": wxT, "biasg": biasg,
            "dtb": dtb, "mb": mbv, "mcolT": mcol, "ident": ident,
        })

    res = _run(nc, in_maps)
    full = []
    for oi in range(6):
        full.append(np.concatenate(
            [res.results[k][f"out{oi}"] for k in range(NCORES)], axis=0))
    return tuple(full)



# revision 37
# speedup vs baseline: 7.3914x; 7.3914x over previous
"""CTLSTM (continuous-time LSTM) Trainium2 kernel.

Strategy (8 NeuronCores, data-parallel over batch):
  - Each core owns 8 of the 64 sequences and runs the full temporal scan.
  - Gate-major layout: gate dim on SBUF partitions (14 tiles of 128),
    batch on the free dim, so all elementwise work is small wide tiles.
  - Host uploads x pre-transposed in bf16; xg = x @ Wx.T + (bx+bh) is
    computed on-device in bf16 and kept resident in SBUF (f32) for the
    whole scan -- no DRAM round-trip.
  - The 8 sequences are split into TWO phase-shifted lanes of 4: while
    lane A runs its elementwise tail, lane B's recurrent matmuls keep
    the PE busy, hiding the cross-engine latency chain.
  - Recurrent matmul per lane-step: 14 gate-tiles x 2 K-chunks of bf16
    stationary Wh tiles against the [128, 4] hidden state.
  - All in-scan activations come from ONE ACT table set (exp_and_others:
    tanh + exp): sigmoid(x) = 0.5 + 0.5*tanh(x/2) (z-gate weights are
    pre-scaled by 2 so z shares the same tanh(x/2) call), and
    softplus(x) = relu(x) + ln1p(exp(-|x|)) with ln1p approximated by a
    cubic polynomial -- no table switches.
  - Only c/c_bar/o/d are written out (fp16, staged gate-major, transposed
    to batch-major via the PE every 8 steps, masked); hn ("befores") and
    afters_h are recomputed on the host from those four, which halves
    the device->host transfer over the tunnel.
  - dt/mask tables are uploaded as single rows and broadcast to 128
    partitions on-device; output zero-buffers are created on-device.
"""

import sys
import numpy as np

B, L_FULL, I, H = 64, 512, 256, 256
NCORES, BC = 8, 8   # cores, sequences per core
NLANE, LB = 2, 4    # lanes per core, sequences per lane
G = 7 * H
NT = 14             # gate tiles of 128

# Tile order (blocks of 128 gate rows): d0,d1, z0,z1, i0,i1, ib0,ib1,
# f0,f1, fb0,fb1, o0,o1.  Original gate offsets in g: i@0, f@256, z@512,
# o@768, d@1024, ib@1280, fb@1536.
PERM_STARTS = [1024, 1152, 512, 640, 0, 128, 1280, 1408, 256, 384,
               1536, 1664, 768, 896]
PERM_ROWS = np.concatenate([np.arange(s, s + 128) for s in PERM_STARTS])
Z_BLOCKS = (2, 3)  # tile indices whose rows get the x2 pre-scale

# ln1p(u) on [0, 1], least-squares fit on a dense grid, degree 3.
_u = np.linspace(0.0, 1.0, 20001)
_c = np.polyfit(_u, np.log1p(_u), 3)[::-1]  # c0..c3
LN1P_C = [float(v) for v in _c] + [0.0, 0.0]

_BUILD_CACHE = {}
DBG_SKIP = set()  # debug: subset of {'pre','chain','mms','flush','pack'}


def _pack_rows(lens, L):
    """Padded packed-row count: max over cores of sum_b (len_b+1),
    rounded up to a multiple of 128."""
    rows = [sum(int(l) + 1 for l in lens[c * BC:(c + 1) * BC])
            for c in range(NCORES)]
    m = max(rows)
    return (m + 127) // 128 * 128


def _balance(lens):
    """Assign sequences to cores so per-core sum(len+1) is balanced
    (greedy LPT).  Returns perm with perm[c*BC+i] = original batch index."""
    order = sorted(range(len(lens)), key=lambda b: -lens[b])
    sums = [0] * NCORES
    counts = [0] * NCORES
    assign = [[] for _ in range(NCORES)]
    for b in order:
        c = min((c for c in range(NCORES) if counts[c] < BC),
                key=lambda c: sums[c])
        assign[c].append(b)
        sums[c] += lens[b] + 1
        counts[c] += 1
    return [b for group in assign for b in group]


def _build(L, lens=None, pack=None, reps=1):
    """Build + schedule the bass module for sequence length L.

    When pack (or lens, from which it is derived) is given, outputs are
    written ragged-packed: per core only sum_b(len_b+1) rows are produced
    (padded to PACK, a multiple of 128, uniform across cores), gathered
    from the padded scratch via indirect DMA; the index table is a
    runtime input, so the build depends only on (L, PACK).
    """
    sys.path.insert(0, "/opt/trn_rl_repo")
    import concourse.bass as bass
    import concourse.tile as tile
    import concourse.mybir as mybir
    from concourse import bacc
    from contextlib import ExitStack

    f32 = mybir.dt.float32
    f16 = mybir.dt.float16
    i32 = mybir.dt.int32
    bf16 = mybir.dt.bfloat16
    AF = mybir.ActivationFunctionType
    OP = mybir.AluOpType

    BCL = BC * L
    NBLK = L // 8          # 8-step staging blocks
    PACK = pack if pack is not None else (
        _pack_rows(lens, L) if lens is not None else None)

    nc = bacc.Bacc("TRN2", target_bir_lowering=False, debug=False,
                   num_devices=NCORES)

    assert PACK is not None
    # Few, fat bindings: each bound tensor costs ~23ms of axon dispatch
    # per call, so everything is fused into 4 inputs and 1 output.
    # win: [whT | wxT] bf16; xin: transposed x bf16;
    # fin (row-major blob, viewed [128, w] on device):
    #   [biasg 128x14 | mcolT 128x2NBLK | ident 128x128 | dtrow L*16]
    NF = 128 * NT + 128 * 2 * NBLK + 128 * 128 + L * 16
    win_in = nc.dram_tensor("win", [128, 2 * 28 * 128], bf16,
                            kind="ExternalInput")
    xin_in = nc.dram_tensor("xin", [128, 2 * BCL], bf16, kind="ExternalInput")
    fin_in = nc.dram_tensor("fin", [1, NF], f32, kind="ExternalInput")
    pidx_in = nc.dram_tensor("pidx", [128, PACK // 128], i32,
                             kind="ExternalInput")
    # c, c_bar, o, d (afters); hn/afters_h are recomputed host-side
    outs = [nc.dram_tensor(f"pad{i}", [BC, L + 1, H], f16) for i in range(4)]
    outp = nc.dram_tensor("outp", [4 * PACK, H], f16, kind="ExternalOutput")

    def fin_seg(off, p, w):
        return fin_in[0:1, off:off + p * w].rearrange(
            "one (p c) -> (one p) c", p=p)

    c0, c1, c2, c3, c4, c5 = LN1P_C

    with tile.TileContext(nc) as tc, ExitStack() as ctx:
        const_pool = ctx.enter_context(tc.tile_pool(name="const", bufs=1))
        whT = const_pool.tile([128, 28 * 128], bf16)
        nc.sync.dma_start(whT[:], win_in[:, 0:28 * 128])
        off = 0
        biasg = const_pool.tile([128, NT], f32)
        nc.sync.dma_start(biasg[:], fin_seg(off, 128, NT))
        off += 128 * NT
        mcol = const_pool.tile([128, 2 * NBLK], f32)
        nc.sync.dma_start(mcol[:], fin_seg(off, 128, 2 * NBLK))
        off += 128 * 2 * NBLK
        ident = const_pool.tile([128, 128], f32)
        nc.sync.dma_start(ident[:], fin_seg(off, 128, 128))
        off += 128 * 128

        # dt table: load one row, broadcast to 128 partitions by
        # doubling SBUF->SBUF DMAs.
        dtb = const_pool.tile([128, L * 16], f32)
        nc.sync.dma_start(dtb[0:1, :], fin_in[0:1, off:off + L * 16])
        k = 1
        while k < 128:
            nc.sync.dma_start(dtb[k:2 * k, :], dtb[0:k, :])
            k *= 2

        # zero out t=0 of every output
        zt0 = const_pool.tile([128, 256], f16)
        nc.vector.memset(zt0[:], 0.0)
        for oi in range(4):
            nc.sync.dma_start(outs[oi][:, 0, :], zt0[0:BC, :])

        # persistent xg buffer: [128, NT*BC*L] f16, t contiguous
        xg_pool = ctx.enter_context(tc.tile_pool(name="xg", bufs=1))
        xg_sb = xg_pool.tile([128, NT * BC * L], f16)

        for _rep in range(reps):
            # ---------- Phase 1: xg = x @ Wx_p.T + bias (bf16 matmul) ----
            with tc.tile_pool(name="xT_pool", bufs=1) as xT_pool, \
                 tc.tile_pool(name="wx_pool", bufs=1) as wx_pool, \
                 tc.tile_pool(name="mm_ps", bufs=4, space="PSUM") as mm_ps:
                wxT = wx_pool.tile([128, 28 * 128], bf16)
                nc.sync.dma_start(wxT[:], win_in[:, 28 * 128:2 * 28 * 128])
                xT = xT_pool.tile([128, 2 * BCL], bf16)
                nc.sync.dma_start(xT[:], xin_in[:])

                if 'pre' in DBG_SKIP:
                    nc.vector.memset(xg_sb[:], 0.0)
                for j in range(0 if 'pre' in DBG_SKIP else NT):
                    for b in range(BC):
                        ps = mm_ps.tile([128, L], f32, tag="ps")
                        nc.tensor.matmul(ps[:], wxT[:, (2 * j) * 128:(2 * j + 1) * 128],
                                         xT[:, 0 * BCL + b * L: 0 * BCL + (b + 1) * L],
                                         start=True, stop=False)
                        nc.tensor.matmul(ps[:], wxT[:, (2 * j + 1) * 128:(2 * j + 2) * 128],
                                         xT[:, 1 * BCL + b * L: 1 * BCL + (b + 1) * L],
                                         start=False, stop=True)
                        dst = xg_sb[:, (j * BC + b) * L:(j * BC + b + 1) * L]
                        if (j * BC + b) % 2 == 0:
                            nc.scalar.activation(dst, ps[:], AF.Identity,
                                                 bias=biasg[:, j:j + 1])
                        else:
                            nc.vector.tensor_scalar(dst, ps[:],
                                                    biasg[:, j:j + 1], None,
                                                    op0=OP.add)

            # ---------- Phase 2: the scan (two phase-shifted lanes) ----------
            # Explicit 2-stage software pipeline: per half-step we emit lane X's
            # recurrent matmuls, then the *previous* half-step's elementwise
            # chain (of the other lane), so the PE stays busy while DVE/ACT run.
            with tc.tile_pool(name="state", bufs=3) as state_pool, \
                 tc.tile_pool(name="gps_d", bufs=3, space="PSUM") as gps_d_pool, \
                 tc.tile_pool(name="tp", bufs=2, space="PSUM") as tp_pool, \
                 tc.tile_pool(name="work", bufs=3) as work_pool, \
                 tc.tile_pool(name="stg", bufs=2) as stg_pool, \
                 tc.tile_pool(name="omask", bufs=3) as omask_pool:

                hn_bf = [None] * NLANE
                cn_half = [None] * NLANE
                for ln in range(NLANE):
                    hn_bf[ln] = state_pool.tile([128, 8], bf16, tag=f"hn_bf{ln}",
                                                name=f"hn_bf{ln}")
                    nc.vector.memset(hn_bf[ln][:], 0.0)
                    cn_half[ln] = state_pool.tile([128, 8], f32, tag=f"cn_half{ln}",
                                                  name=f"cn_half{ln}")
                    nc.vector.memset(cn_half[ln][:], 0.0)

                xgv = xg_sb[:].rearrange("p (j b t) -> p j b t", j=NT, b=BC)
                stg = {}

                def emit_mms(ln, t):
                    g_all = gps_d_pool.tile([128, 56], f32, tag="g_all",
                                            name=f"g_all{ln}")
                    if 'mms' in DBG_SKIP:
                        nc.vector.memset(g_all[:], 0.0)
                        return g_all
                    hb = hn_bf[ln]
                    for j in range(NT):
                        dst = g_all[:, j * 4:(j + 1) * 4]
                        for k in range(2):
                            nc.tensor.matmul(
                                dst,
                                whT[:, (2 * j + k) * 128:(2 * j + k + 1) * 128],
                                hb[:, k * LB:(k + 1) * LB],
                                start=(k == 0), stop=(k == 1))
                    return g_all

                def make_chain(ln, t, g_all):
                    kappa, blk = t % 8, t // 8
                    tsl = slice(t * 16 + ln * 8, t * 16 + ln * 8 + 8)
                    bsl = slice(ln * LB, (ln + 1) * LB)

                    def chain():
                        if kappa == 0:
                            for nm in ("c", "cb", "o", "d"):
                                stg[(nm, ln)] = stg_pool.tile(
                                    [128, 64], f32, tag=f"stg_{nm}{ln}",
                                    name=f"stg_{nm}{ln}")
                        sl = slice(kappa * 8, kappa * 8 + 8)
                        xg_all = xgv[:, :, bsl, t]

                        gfull = work_pool.tile([128, 56], f32, tag=f"gf{ln}",
                                               name=f"gf{ln}")
                        nc.vector.tensor_tensor(
                            gfull[:].rearrange("p (j b) -> p j b", j=14),
                            g_all[:].rearrange("p (j b) -> p j b", j=14),
                            xg_all, op=OP.add)
                        gd = gfull[:, 0:8]

                        # --- d path: d = relu(gd) + ln1p(exp(-|gd|)) ---
                        ga = work_pool.tile([128, 8], f32, tag=f"ga{ln}",
                                            name=f"ga{ln}")
                        nc.vector.scalar_tensor_tensor(ga[:], gd, -1.0, gd,
                                                       op0=OP.mult, op1=OP.max)
                        uu = work_pool.tile([128, 8], f32, tag=f"uu{ln}",
                                            name=f"uu{ln}")
                        nc.scalar.activation(uu[:], ga[:], AF.Exp, scale=-1.0)
                        pa = work_pool.tile([128, 8], f32, tag=f"pa{ln}",
                                            name=f"pa{ln}")
                        nc.vector.tensor_scalar(pa[:], uu[:], c3, None, op0=OP.mult)
                        pb = work_pool.tile([128, 8], f32, tag=f"pb{ln}",
                                            name=f"pb{ln}")
                        nc.vector.scalar_tensor_tensor(pb[:], pa[:], c2, uu[:],
                                                       op0=OP.add, op1=OP.mult)
                        nc.vector.scalar_tensor_tensor(pb[:], pb[:], c1, uu[:],
                                                       op0=OP.add, op1=OP.mult)
                        # d = max(gd, 0) + poly   (c0 ~ 1e-5 dropped)
                        nc.vector.scalar_tensor_tensor(stg[("d", ln)][:, sl],
                                                       gd, 0.0, pb[:],
                                                       op0=OP.max, op1=OP.add)
                        md = work_pool.tile([128, 8], f32, tag=f"md{ln}",
                                            name=f"md{ln}")
                        nc.vector.tensor_tensor(md[:], stg[("d", ln)][:, sl],
                                                dtb[:, tsl], op=OP.mult)
                        et = work_pool.tile([128, 8], f32, tag=f"et{ln}",
                                            name=f"et{ln}")
                        nc.scalar.activation(et[:], md[:], AF.Exp, scale=-1.0)

                        # --- z + sigmoid gates ---
                        gt = work_pool.tile([128, 48], f32, tag=f"gt{ln}",
                                            name=f"gt{ln}")
                        nc.scalar.activation(gt[:], gfull[:, 8:56], AF.Tanh,
                                             scale=0.5)

                        iz_i = work_pool.tile([128, 8], f32, tag=f"iz_i{ln}",
                                              name=f"iz_i{ln}")
                        nc.vector.scalar_tensor_tensor(iz_i[:], gt[:, 8:16], 1.0,
                                                       gt[:, 0:8], op0=OP.add,
                                                       op1=OP.mult)
                        iz_ib = work_pool.tile([128, 8], f32, tag=f"iz_ib{ln}",
                                               name=f"iz_ib{ln}")
                        nc.vector.scalar_tensor_tensor(iz_ib[:], gt[:, 16:24], 1.0,
                                                       gt[:, 0:8], op0=OP.add,
                                                       op1=OP.mult)
                        fc_f = work_pool.tile([128, 8], f32, tag=f"fc_f{ln}",
                                              name=f"fc_f{ln}")
                        nc.vector.scalar_tensor_tensor(fc_f[:], gt[:, 24:32], 1.0,
                                                       cn_half[ln][:], op0=OP.add,
                                                       op1=OP.mult)
                        fc_fb = work_pool.tile([128, 8], f32, tag=f"fc_fb{ln}",
                                               name=f"fc_fb{ln}")
                        nc.vector.scalar_tensor_tensor(fc_fb[:], gt[:, 32:40], 1.0,
                                                       cn_half[ln][:], op0=OP.add,
                                                       op1=OP.mult)
                        nc.vector.scalar_tensor_tensor(stg[("c", ln)][:, sl],
                                                       iz_i[:], 0.5, fc_f[:],
                                                       op0=OP.mult, op1=OP.add)
                        nc.vector.scalar_tensor_tensor(stg[("cb", ln)][:, sl],
                                                       iz_ib[:], 0.5, fc_fb[:],
                                                       op0=OP.mult, op1=OP.add)
                        nc.vector.tensor_scalar(stg[("o", ln)][:, sl], gt[:, 40:48],
                                                1.0, 0.5, op0=OP.add, op1=OP.mult)

                        # --- decay + new state ---
                        dd = work_pool.tile([128, 8], f32, tag=f"dd{ln}",
                                            name=f"dd{ln}")
                        nc.vector.tensor_tensor(dd[:], stg[("c", ln)][:, sl],
                                                stg[("cb", ln)][:, sl],
                                                op=OP.subtract)
                        de = work_pool.tile([128, 8], f32, tag=f"de{ln}",
                                            name=f"de{ln}")
                        nc.vector.tensor_tensor(de[:], dd[:], et[:], op=OP.mult)
                        ctt = work_pool.tile([128, 8], f32, tag=f"ctt{ln}",
                                             name=f"ctt{ln}")
                        nc.vector.tensor_tensor(ctt[:], de[:],
                                                stg[("cb", ln)][:, sl], op=OP.add)
                        tct = work_pool.tile([128, 8], f32, tag=f"tct{ln}",
                                             name=f"tct{ln}")
                        nc.scalar.activation(tct[:], ctt[:], AF.Tanh)
                        # state stays unmasked: outputs are masked at flush,
                        # and post-seq_len state never feeds a valid output.
                        hn_bf[ln] = state_pool.tile([128, 8], bf16,
                                                    tag=f"hn_bf{ln}",
                                                    name=f"hn_bf{ln}")
                        nc.vector.tensor_tensor(hn_bf[ln][:],
                                                stg[("o", ln)][:, sl],
                                                tct[:], op=OP.mult)
                        cn_half[ln] = state_pool.tile([128, 8], f32,
                                                      tag=f"cn_half{ln}",
                                                      name=f"cn_half{ln}")
                        nc.vector.tensor_scalar(cn_half[ln][:], ctt[:], 0.5,
                                                None, op0=OP.mult)

                        if kappa == 7 and 'flush' not in DBG_SKIP:
                            emit_flush(ln, blk)
                    return chain

                def emit_flush(ln, blk):
                    mcol_ap = mcol[:, blk * 2 + ln: blk * 2 + ln + 1]

                    def out_view(oi):
                        return outs[oi][ln * LB:(ln + 1) * LB,
                                        blk * 8 + 1: blk * 8 + 9, :] \
                            .rearrange("b t (c h) -> t c b h", c=2)

                    for oi, nm in ((0, "c"), (1, "cb"), (2, "o"), (3, "d")):
                        tp = tp_pool.tile([128, 128], f32, tag="tp",
                                          name=f"tp_{nm}")
                        nc.tensor.transpose(tp[0:64, :], stg[(nm, ln)][:],
                                            ident[:])
                        om = omask_pool.tile([128, 128], f16, tag=f"om_{nm}",
                                             name=f"om_{nm}")
                        nc.vector.tensor_scalar_mul(om[0:64, :], tp[0:64, :],
                                                    mcol_ap[0:64])
                        nc.sync.dma_start(out_view(oi), om[0:64, :])

                pending = []
                for t in range(L):
                    for ln in range(NLANE):
                        g_d = emit_mms(ln, t)
                        if 'chain' not in DBG_SKIP:
                            if pending:
                                pending.pop(0)()
                            pending.append(make_chain(ln, t, g_d))
                while pending:
                    pending.pop(0)()

            # ---------- Phase 3: ragged pack via indirect gather ----------
            if 'pack' not in DBG_SKIP:
                with tc.tile_pool(name="pk_idx", bufs=1) as pk_idx_pool, \
                     tc.tile_pool(name="pk_stage", bufs=6) as pk_stage_pool:
                    pidx = pk_idx_pool.tile([128, PACK // 128], i32)
                    nc.sync.dma_start(pidx[:], pidx_in[:])
                    for oi in range(4):
                        src = outs[oi][:].rearrange("b t h -> (b t) h")
                        for ch in range(PACK // 128):
                            stage = pk_stage_pool.tile([128, H], f16,
                                                       tag="pkst")
                            nc.gpsimd.indirect_dma_start(
                                out=stage[:], out_offset=None,
                                in_=src,
                                in_offset=bass.IndirectOffsetOnAxis(
                                    ap=pidx[:, ch:ch + 1], axis=0))
                            nc.sync.dma_start(
                                outp[oi * PACK + ch * 128:
                                     oi * PACK + (ch + 1) * 128, :],
                                stage[:])

    nc.finalize()
    # The module never reads partition_id; dropping its allocation saves
    # one per-call binding RPC.  Fall back silently if not removable.
    try:
        import concourse.mybir as _mybir
        f0 = nc.m.functions[0]
        for a in list(f0.allocations):
            if (isinstance(a, _mybir.MemoryLocationSet) and a.memorylocations
                    and a.memorylocations[0].name == "partition_id"):
                f0.allocations.remove(a)
                nc.partition_id_tensor = None
                break
    except Exception:
        pass
    return nc


def _prep_shared(Wx, bx, Wh, bh):
    import ml_dtypes
    Wh_p = Wh[PERM_ROWS].astype(np.float32).copy()
    Wx_p = Wx[PERM_ROWS].astype(np.float32).copy()
    bias_p = (bx + bh)[PERM_ROWS].astype(np.float32).copy()
    for zb in Z_BLOCKS:
        Wh_p[zb * 128:(zb + 1) * 128] *= 2.0
        Wx_p[zb * 128:(zb + 1) * 128] *= 2.0
        bias_p[zb * 128:(zb + 1) * 128] *= 2.0

    win = np.zeros((128, 2 * 28 * 128), dtype=ml_dtypes.bfloat16)
    for j in range(NT):
        for k in range(2):
            s = (2 * j + k) * 128
            win[:, s:s + 128] = Wh_p[j * 128:(j + 1) * 128,
                                     k * 128:(k + 1) * 128].T
            win[:, 28 * 128 + s:28 * 128 + s + 128] = \
                Wx_p[j * 128:(j + 1) * 128, k * 128:(k + 1) * 128].T
    biasg = np.zeros((128, NT), dtype=np.float32)
    for j in range(NT):
        biasg[:, j] = bias_p[j * 128:(j + 1) * 128]
    return win, biasg


def _prep_core(xc, dtc, slc, L):
    import ml_dtypes
    x_rows = xc.reshape(BC * L, I).astype(np.float32)
    xTf = x_rows.T  # [I, BCL]
    xT = np.concatenate([xTf[:128], xTf[128:]], axis=1).astype(ml_dtypes.bfloat16)
    t_idx = np.arange(L)
    m = (t_idx[None, :] < slc[:, None]).astype(np.float32)  # [BC, L]
    dt2 = dtc[:, :, 0].astype(np.float32)  # [BC, L]
    # [1, L*16]: column t*16 + lane*8 + c*4 + b' -> value for (b, t)
    # where b = lane*4 + b'
    col_dt = np.empty((L, 2, 2, LB), np.float32)
    for ln in range(NLANE):
        for c in range(2):
            col_dt[:, ln, c, :] = dt2[ln * LB:(ln + 1) * LB, :].T
    dtrow = col_dt.reshape(L * 16)
    # mcolT [128, 2*NBLK]: partition p = kappa*8 + c*4 + b', col = blk*2+lane
    NBLK = L // 8
    mcol = np.zeros((128, 2 * NBLK), dtype=np.float32)
    for blk in range(NBLK):
        for ln in range(NLANE):
            v = m[ln * LB:(ln + 1) * LB, blk * 8:blk * 8 + 8]  # [b', kappa]
            col = np.repeat(v.T[:, None, :], 2, axis=1)  # [kappa, c, b']
            mcol[0:64, blk * 2 + ln] = col.reshape(64)
    return xT, dtrow, mcol


class _CachedRunner:
    """Build the sharded jitted executable once; reuse across calls so the
    NEFF is loaded on the devices a single time.  Output zero-buffers are
    created on-device (never uploaded)."""

    def __init__(self, nc):
        sys.path.insert(0, "/opt/trn_rl_repo")
        import jax
        import jax.numpy as jnp
        import numpy as _np
        from jax.sharding import Mesh, PartitionSpec, NamedSharding
        from jax.experimental.shard_map import shard_map
        from concourse import mybir
        from concourse.bass2jax import _bass_exec_p, partition_id_tensor, \
            install_neuronx_cc_hook
        install_neuronx_cc_hook()
        self.jax = jax
        partition_name = (nc.partition_id_tensor.name
                          if nc.partition_id_tensor else None)
        in_names, out_names, out_avals = [], [], []
        for alloc in nc.m.functions[0].allocations:
            if not isinstance(alloc, mybir.MemoryLocationSet):
                continue
            name = alloc.memorylocations[0].name
            if alloc.kind == "ExternalInput":
                if name != partition_name:
                    in_names.append(name)
            elif alloc.kind == "ExternalOutput":
                out_names.append(name)
                shape = tuple(alloc.tensor_shape)
                dtype = mybir.dt.np(alloc.dtype)
                out_avals.append(jax.core.ShapedArray(shape, dtype))
        self.n_params = len(in_names)
        self.in_names = list(in_names)
        self.out_names = out_names
        self.out_avals = out_avals
        in_names_all = in_names + out_names
        if partition_name is not None:
            in_names_all.append(partition_name)

        def _body(*args):
            operands = list(args)
            if partition_name is not None:
                operands.append(partition_id_tensor())
            outs = _bass_exec_p.bind(
                *operands, out_avals=tuple(out_avals),
                in_names=tuple(in_names_all), out_names=tuple(out_names),
                lowering_input_output_aliases=(), sim_require_finite=True,
                sim_require_nnan=True, nc=nc)
            return tuple(outs)

        n_outs = len(out_avals)
        devices = jax.devices()[:NCORES]
        mesh = Mesh(_np.asarray(devices), ("core",))
        sharding = NamedSharding(mesh, PartitionSpec("core"))
        self._sharding = sharding
        in_specs = (PartitionSpec("core"),) * (self.n_params + n_outs)
        out_specs = (PartitionSpec("core"),) * n_outs
        donate = tuple(range(self.n_params, self.n_params + n_outs))
        self.sharded = jax.jit(
            shard_map(_body, mesh=mesh, in_specs=in_specs,
                      out_specs=out_specs, check_rep=False),
            donate_argnums=donate, keep_unused=True)
        # zero output buffers are created on-device (never uploaded)
        self._zeros_fn = jax.jit(
            lambda: tuple(
                jnp.zeros((NCORES * a.shape[0], *a.shape[1:]), a.dtype)
                for a in out_avals),
            out_shardings=(sharding,) * n_outs)

    # inputs that usually don't change between calls -- keep them
    # device-resident across calls keyed by content digest.
    _STABLE = {"win", "fin", "pidx"}

    def __call__(self, in_maps):
        import numpy as _np
        import hashlib
        if not hasattr(self, "_stable_cache"):
            self._stable_cache = {}
            self._prev_out = None
        concat_in = []
        for i, name in enumerate(self.in_names):
            arrs = [_np.asarray(m[name]) for m in in_maps]
            cat = _np.concatenate(arrs, axis=0)
            if name in self._STABLE:
                dig = hashlib.blake2b(cat.tobytes(), digest_size=16).digest()
                hit = self._stable_cache.get(name)
                if hit is not None and hit[0] == dig:
                    concat_in.append(hit[1])
                    continue
                dev = self.jax.device_put(cat, self._sharding)
                self._stable_cache[name] = (dig, dev)
                concat_in.append(dev)
            else:
                concat_in.append(cat)
        # recycle last call's output buffers as this call's donated
        # "zero" buffers -- their initial content is never observed.
        zeros = self._prev_out if self._prev_out is not None \
            else self._zeros_fn()
        out_arrs = self.sharded(*concat_in, *zeros)
        fetched = self.jax.device_get(list(out_arrs))
        self._prev_out = tuple(out_arrs)
        return {name: _np.asarray(fetched[i]).reshape(
                    NCORES, *self.out_avals[i].shape)
                for i, name in enumerate(self.out_names)}


_RUNNER_CACHE = {}


def kernel(x, delta_t, seq_lens, Wx, bx, Wh, bh, _L=None):
    L = _L if _L is not None else x.shape[1]
    x = np.asarray(x)
    delta_t = np.asarray(delta_t)
    seq_lens = np.asarray(seq_lens)
    lens0 = tuple(int(v) for v in seq_lens)
    perm = _balance(lens0)  # perm[c*BC+i] = original batch index
    lens = tuple(lens0[p] for p in perm)
    PACK = _pack_rows(lens, L)
    key = (L, PACK)
    if key not in _BUILD_CACHE:
        _BUILD_CACHE[key] = _build(L, pack=PACK)
    nc = _BUILD_CACHE[key]
    rkey = id(nc)
    if rkey not in _RUNNER_CACHE:
        _RUNNER_CACHE[rkey] = _CachedRunner(nc)
    runner = _RUNNER_CACHE[rkey]

    win, biasg = _prep_shared(np.asarray(Wx), np.asarray(bx),
                              np.asarray(Wh), np.asarray(bh))
    ident = np.eye(128, dtype=np.float32)

    in_maps = []
    for k in range(NCORES):
        sel = perm[k * BC:(k + 1) * BC]
        xT, dtrow, mcol = _prep_core(x[sel], delta_t[sel], seq_lens[sel], L)
        fin = np.concatenate([biasg.ravel(), mcol.ravel(), ident.ravel(),
                              dtrow])[None, :].astype(np.float32)
        # packed-row -> padded-flat-row index table, [128, PACK//128]
        idx = np.zeros(PACK, np.int32)
        pos = 0
        for b in range(BC):
            n = lens[k * BC + b] + 1
            idx[pos:pos + n] = b * (L + 1) + np.arange(n)
            pos += n
        pidx = np.ascontiguousarray(idx.reshape(PACK // 128, 128).T)
        in_maps.append({"win": win, "xin": xT, "fin": fin, "pidx": pidx})

    res = runner(in_maps)
    allout = res["outp"]  # [NCORES, 4*PACK, H] f16
    pk = [allout[:, i * PACK:(i + 1) * PACK] for i in range(4)]

    # host-side: reconstruct befores (hn) + afters_h on the packed rows,
    # then scatter all six outputs into full-size zero arrays.
    from concurrent.futures import ThreadPoolExecutor
    full = [np.zeros((B, L + 1, H), np.float32) for _ in range(6)]

    def _post_core(k):
        rows = sum(v + 1 for v in lens[k * BC:(k + 1) * BC])
        c = pk[0][k][:rows].astype(np.float32)
        cb = pk[1][k][:rows].astype(np.float32)
        o = pk[2][k][:rows].astype(np.float32)
        d = pk[3][k][:rows].astype(np.float32)
        dtp = np.zeros((rows, 1), np.float32)
        pos = 0
        for b in range(BC):
            n = lens[k * BC + b] + 1
            dtp[pos + 1:pos + n, 0] = delta_t[perm[k * BC + b], 0:n - 1, 0]
            pos += n
        ah = o * np.tanh(c)
        ct = cb + (c - cb) * np.exp(-d * dtp)
        bef = o * np.tanh(ct)
        arrs = (bef, ah, c, cb, o, d)
        pos = 0
        for b in range(BC):
            n = lens[k * BC + b] + 1
            gb = perm[k * BC + b]
            for i in range(6):
                full[i][gb, 0:n] = arrs[i][pos:pos + n]
            pos += n

    with ThreadPoolExecutor(NCORES) as ex:
        list(ex.map(_post_core, range(NCORES)))
    return tuple(full)


# revision 46
# speedup vs baseline: 9.4268x; 1.2754x over previous
"""CTLSTM (continuous-time LSTM) Trainium2 kernel.

Strategy (8 NeuronCores, data-parallel over batch):
  - Each core owns 8 of the 64 sequences and runs the full temporal scan.
  - Gate-major layout: gate dim on SBUF partitions (14 tiles of 128),
    batch on the free dim, so all elementwise work is small wide tiles.
  - Host uploads x pre-transposed in bf16; xg = x @ Wx.T + (bx+bh) is
    computed on-device in bf16 and kept resident in SBUF (f32) for the
    whole scan -- no DRAM round-trip.
  - The 8 sequences are split into TWO phase-shifted lanes of 4: while
    lane A runs its elementwise tail, lane B's recurrent matmuls keep
    the PE busy, hiding the cross-engine latency chain.
  - Recurrent matmul per lane-step: 14 gate-tiles x 2 K-chunks of bf16
    stationary Wh tiles against the [128, 4] hidden state.
  - All in-scan activations come from ONE ACT table set (exp_and_others:
    tanh + exp): sigmoid(x) = 0.5 + 0.5*tanh(x/2) (z-gate weights are
    pre-scaled by 2 so z shares the same tanh(x/2) call), and
    softplus(x) = relu(x) + ln1p(exp(-|x|)) with ln1p approximated by a
    cubic polynomial -- no table switches.
  - Only c/c_bar/o/d are written out (fp16, staged gate-major, transposed
    to batch-major via the PE every 8 steps, masked); hn ("befores") and
    afters_h are recomputed on the host from those four, which halves
    the device->host transfer over the tunnel.
  - dt/mask tables are uploaded as single rows and broadcast to 128
    partitions on-device; output zero-buffers are created on-device.
"""

import sys
import numpy as np

B, L_FULL, I, H = 64, 512, 256, 256
NCORES, BC = 8, 8   # cores, sequences per core
NLANE, LB = 2, 4    # lanes per core, sequences per lane
G = 7 * H
NT = 14             # gate tiles of 128

# Tile order (blocks of 128 gate rows): d0,d1, z0,z1, i0,i1, ib0,ib1,
# f0,f1, fb0,fb1, o0,o1.  Original gate offsets in g: i@0, f@256, z@512,
# o@768, d@1024, ib@1280, fb@1536.
PERM_STARTS = [1024, 1152, 512, 640, 0, 128, 1280, 1408, 256, 384,
               1536, 1664, 768, 896]
PERM_ROWS = np.concatenate([np.arange(s, s + 128) for s in PERM_STARTS])
Z_BLOCKS = (2, 3)  # tile indices whose rows get the x2 pre-scale

# ln1p(u) on [0, 1], least-squares fit on a dense grid, degree 3.
_u = np.linspace(0.0, 1.0, 20001)
_c = np.polyfit(_u, np.log1p(_u), 3)[::-1]  # c0..c3
LN1P_C = [float(v) for v in _c] + [0.0, 0.0]

_BUILD_CACHE = {}
DBG_SKIP = set()  # debug: subset of {'pre','chain','mms','flush','pack'}


def _pack_rows(lens, L):
    """Padded packed-row count: max over cores of sum_b (len_b+1),
    rounded up to a multiple of 128."""
    rows = [sum(int(l) + 1 for l in lens[c * BC:(c + 1) * BC])
            for c in range(NCORES)]
    m = max(rows)
    return (m + 127) // 128 * 128


def _balance(lens):
    """Assign sequences to cores so per-core sum(len+1) is balanced
    (greedy LPT).  Returns perm with perm[c*BC+i] = original batch index."""
    order = sorted(range(len(lens)), key=lambda b: -lens[b])
    sums = [0] * NCORES
    counts = [0] * NCORES
    assign = [[] for _ in range(NCORES)]
    for b in order:
        c = min((c for c in range(NCORES) if counts[c] < BC),
                key=lambda c: sums[c])
        assign[c].append(b)
        sums[c] += lens[b] + 1
        counts[c] += 1
    return [b for group in assign for b in group]


def _build(L, lens=None, pack=None, poslen=None, reps=1):
    """Build + schedule the bass module for sequence length L.

    When pack (or lens, from which it is derived) is given, outputs are
    written ragged-packed: per core only sum_b(len_b+1) rows are produced
    (padded to PACK, a multiple of 128, uniform across cores), gathered
    from the padded scratch via indirect DMA; the index table is a
    runtime input, so the build depends only on (L, PACK).
    """
    sys.path.insert(0, "/opt/trn_rl_repo")
    import concourse.bass as bass
    import concourse.tile as tile
    import concourse.mybir as mybir
    from concourse import bacc
    from contextlib import ExitStack

    f32 = mybir.dt.float32
    f16 = mybir.dt.float16
    i32 = mybir.dt.int32
    bf16 = mybir.dt.bfloat16
    AF = mybir.ActivationFunctionType
    OP = mybir.AluOpType

    BCL = BC * L
    NBLK = L // 8          # 8-step staging blocks
    PACK = pack if pack is not None else (
        _pack_rows(lens, L) if lens is not None else None)
    if poslen is None:
        poslen = (L,) * BC
    PACKX = sum(poslen)
    XOFF = [0] * BC
    for b in range(1, BC):
        XOFF[b] = XOFF[b - 1] + poslen[b - 1]

    nc = bacc.Bacc("TRN2", target_bir_lowering=False, debug=False,
                   num_devices=NCORES)

    assert PACK is not None
    # Few, fat bindings: each bound tensor costs ~23ms of axon dispatch
    # per call, so everything is fused into 4 inputs and 1 output.
    # win: [whT | wxT] bf16; xin: transposed x bf16;
    # fin (row-major blob, viewed [128, w] on device):
    #   [biasg 128x14 | mcolT 128x2NBLK | ident 128x128 | dtrow L*16]
    NF = 128 * NT + 128 * 2 * NBLK + 128 * 128 + L * 16
    win_in = nc.dram_tensor("win", [128, 2 * 28 * 128], bf16,
                            kind="ExternalInput")
    xin_in = nc.dram_tensor("xin", [128, 2 * PACKX], bf16,
                            kind="ExternalInput")
    fin_in = nc.dram_tensor("fin", [1, NF], f32, kind="ExternalInput")
    pidx_in = nc.dram_tensor("pidx", [128, PACK // 128], i32,
                             kind="ExternalInput")
    # c, c_bar, o, d (afters); hn/afters_h are recomputed host-side
    outs = [nc.dram_tensor(f"pad{i}", [BC, L + 1, H], f16) for i in range(4)]
    outp = nc.dram_tensor("outp", [4 * PACK, H], f16, kind="ExternalOutput")

    def fin_seg(off, p, w):
        return fin_in[0:1, off:off + p * w].rearrange(
            "one (p c) -> (one p) c", p=p)

    c0, c1, c2, c3, c4, c5 = LN1P_C

    with tile.TileContext(nc) as tc, ExitStack() as ctx:
        const_pool = ctx.enter_context(tc.tile_pool(name="const", bufs=1))
        whT = const_pool.tile([128, 28 * 128], bf16)
        nc.sync.dma_start(whT[:], win_in[:, 0:28 * 128])
        off = 0
        biasg = const_pool.tile([128, NT], f32)
        nc.sync.dma_start(biasg[:], fin_seg(off, 128, NT))
        off += 128 * NT
        mcol = const_pool.tile([128, 2 * NBLK], f32)
        nc.sync.dma_start(mcol[:], fin_seg(off, 128, 2 * NBLK))
        off += 128 * 2 * NBLK
        ident = const_pool.tile([128, 128], f32)
        nc.sync.dma_start(ident[:], fin_seg(off, 128, 128))
        off += 128 * 128

        # dt table: load one row, broadcast to 128 partitions by
        # doubling SBUF->SBUF DMAs.
        dtb = const_pool.tile([128, L * 16], f32)
        nc.sync.dma_start(dtb[0:1, :], fin_in[0:1, off:off + L * 16])
        k = 1
        while k < 128:
            nc.sync.dma_start(dtb[k:2 * k, :], dtb[0:k, :])
            k *= 2

        # zero out t=0 of every output
        zt0 = const_pool.tile([128, 256], f16)
        nc.vector.memset(zt0[:], 0.0)
        for oi in range(4):
            nc.sync.dma_start(outs[oi][:, 0, :], zt0[0:BC, :])

        # persistent xg buffer: [128, NT*BC*L] f16, t contiguous
        xg_pool = ctx.enter_context(tc.tile_pool(name="xg", bufs=1))
        xg_sb = xg_pool.tile([128, NT * BC * L], f16)

        for _rep in range(reps):
            # ---------- Phase 1: xg = x @ Wx_p.T + bias (bf16 matmul) ----
            with tc.tile_pool(name="xT_pool", bufs=1) as xT_pool, \
                 tc.tile_pool(name="wx_pool", bufs=1) as wx_pool, \
                 tc.tile_pool(name="mm_ps", bufs=4, space="PSUM") as mm_ps:
                wxT = wx_pool.tile([128, 28 * 128], bf16)
                nc.sync.dma_start(wxT[:], win_in[:, 28 * 128:2 * 28 * 128])
                xT = xT_pool.tile([128, 2 * PACKX], bf16)
                nc.sync.dma_start(xT[:], xin_in[:])

                if 'pre' in DBG_SKIP:
                    nc.vector.memset(xg_sb[:], 0.0)
                for j in range(0 if 'pre' in DBG_SKIP else NT):
                    for b in range(BC):
                        n = poslen[b]
                        ps = mm_ps.tile([128, L], f32, tag="ps")
                        nc.tensor.matmul(ps[:, :n],
                                         wxT[:, (2 * j) * 128:(2 * j + 1) * 128],
                                         xT[:, XOFF[b]:XOFF[b] + n],
                                         start=True, stop=False)
                        nc.tensor.matmul(ps[:, :n],
                                         wxT[:, (2 * j + 1) * 128:(2 * j + 2) * 128],
                                         xT[:, PACKX + XOFF[b]:PACKX + XOFF[b] + n],
                                         start=False, stop=True)
                        dst = xg_sb[:, (j * BC + b) * L:(j * BC + b) * L + n]
                        if (j * BC + b) % 2 == 0:
                            nc.scalar.activation(dst, ps[:, :n], AF.Identity,
                                                 bias=biasg[:, j:j + 1])
                        else:
                            nc.vector.tensor_scalar(dst, ps[:, :n],
                                                    biasg[:, j:j + 1], None,
                                                    op0=OP.add)

            # ---------- Phase 2: the scan (two phase-shifted lanes) ----------
            # Explicit 2-stage software pipeline: per half-step we emit lane X's
            # recurrent matmuls, then the *previous* half-step's elementwise
            # chain (of the other lane), so the PE stays busy while DVE/ACT run.
            with tc.tile_pool(name="state", bufs=3) as state_pool, \
                 tc.tile_pool(name="gps_d", bufs=3, space="PSUM") as gps_d_pool, \
                 tc.tile_pool(name="tp", bufs=2, space="PSUM") as tp_pool, \
                 tc.tile_pool(name="work", bufs=3) as work_pool, \
                 tc.tile_pool(name="stg", bufs=2) as stg_pool, \
                 tc.tile_pool(name="omask", bufs=3) as omask_pool:

                hn_bf = [None] * NLANE
                cn_half = [None] * NLANE
                for ln in range(NLANE):
                    hn_bf[ln] = state_pool.tile([128, 8], bf16, tag=f"hn_bf{ln}",
                                                name=f"hn_bf{ln}")
                    nc.vector.memset(hn_bf[ln][:], 0.0)
                    cn_half[ln] = state_pool.tile([128, 8], f32, tag=f"cn_half{ln}",
                                                  name=f"cn_half{ln}")
                    nc.vector.memset(cn_half[ln][:], 0.0)

                xgv = xg_sb[:].rearrange("p (j b t) -> p j b t", j=NT, b=BC)
                stg = {}

                def emit_mms(ln, t):
                    g_all = gps_d_pool.tile([128, 56], f32, tag="g_all",
                                            name=f"g_all{ln}")
                    if 'mms' in DBG_SKIP:
                        nc.vector.memset(g_all[:], 0.0)
                        return g_all
                    hb = hn_bf[ln]
                    for j in range(NT):
                        dst = g_all[:, j * 4:(j + 1) * 4]
                        for k in range(2):
                            nc.tensor.matmul(
                                dst,
                                whT[:, (2 * j + k) * 128:(2 * j + k + 1) * 128],
                                hb[:, k * LB:(k + 1) * LB],
                                start=(k == 0), stop=(k == 1))
                    return g_all

                def make_chain(ln, t, g_all):
                    kappa, blk = t % 8, t // 8
                    tsl = slice(t * 16 + ln * 8, t * 16 + ln * 8 + 8)
                    bsl = slice(ln * LB, (ln + 1) * LB)

                    def chain():
                        if kappa == 0:
                            for nm in ("c", "cb", "o", "d"):
                                stg[(nm, ln)] = stg_pool.tile(
                                    [128, 64], f32, tag=f"stg_{nm}{ln}",
                                    name=f"stg_{nm}{ln}")
                        sl = slice(kappa * 8, kappa * 8 + 8)
                        xg_all = xgv[:, :, bsl, t]

                        gfull = work_pool.tile([128, 56], f32, tag=f"gf{ln}",
                                               name=f"gf{ln}")
                        nc.vector.tensor_tensor(
                            gfull[:].rearrange("p (j b) -> p j b", j=14),
                            g_all[:].rearrange("p (j b) -> p j b", j=14),
                            xg_all, op=OP.add)
                        gd = gfull[:, 0:8]

                        # --- d path: d = relu(gd) + ln1p(exp(-|gd|)) ---
                        ga = work_pool.tile([128, 8], f32, tag=f"ga{ln}",
                                            name=f"ga{ln}")
                        nc.vector.scalar_tensor_tensor(ga[:], gd, -1.0, gd,
                                                       op0=OP.mult, op1=OP.max)
                        uu = work_pool.tile([128, 8], f32, tag=f"uu{ln}",
                                            name=f"uu{ln}")
                        nc.scalar.activation(uu[:], ga[:], AF.Exp, scale=-1.0)
                        pa = work_pool.tile([128, 8], f32, tag=f"pa{ln}",
                                            name=f"pa{ln}")
                        nc.vector.tensor_scalar(pa[:], uu[:], c3, None, op0=OP.mult)
                        pb = work_pool.tile([128, 8], f32, tag=f"pb{ln}",
                                            name=f"pb{ln}")
                        nc.vector.scalar_tensor_tensor(pb[:], pa[:], c2, uu[:],
                                                       op0=OP.add, op1=OP.mult)
                        nc.vector.scalar_tensor_tensor(pb[:], pb[:], c1, uu[:],
                                                       op0=OP.add, op1=OP.mult)
                        # d = max(gd, 0) + poly   (c0 ~ 1e-5 dropped)
                        nc.vector.scalar_tensor_tensor(stg[("d", ln)][:, sl],
                                                       gd, 0.0, pb[:],
                                                       op0=OP.max, op1=OP.add)
                        md = work_pool.tile([128, 8], f32, tag=f"md{ln}",
                                            name=f"md{ln}")
                        nc.vector.tensor_tensor(md[:], stg[("d", ln)][:, sl],
                                                dtb[:, tsl], op=OP.mult)
                        et = work_pool.tile([128, 8], f32, tag=f"et{ln}",
                                            name=f"et{ln}")
                        nc.scalar.activation(et[:], md[:], AF.Exp, scale=-1.0)

                        # --- z + sigmoid gates ---
                        gt = work_pool.tile([128, 48], f32, tag=f"gt{ln}",
                                            name=f"gt{ln}")
                        nc.scalar.activation(gt[:], gfull[:, 8:56], AF.Tanh,
                                             scale=0.5)

                        iz_i = work_pool.tile([128, 8], f32, tag=f"iz_i{ln}",
                                              name=f"iz_i{ln}")
                        nc.vector.scalar_tensor_tensor(iz_i[:], gt[:, 8:16], 1.0,
                                                       gt[:, 0:8], op0=OP.add,
                                                       op1=OP.mult)
                        iz_ib = work_pool.tile([128, 8], f32, tag=f"iz_ib{ln}",
                                               name=f"iz_ib{ln}")
                        nc.vector.scalar_tensor_tensor(iz_ib[:], gt[:, 16:24], 1.0,
                                                       gt[:, 0:8], op0=OP.add,
                                                       op1=OP.mult)
                        fc_f = work_pool.tile([128, 8], f32, tag=f"fc_f{ln}",
                                              name=f"fc_f{ln}")
                        nc.vector.scalar_tensor_tensor(fc_f[:], gt[:, 24:32], 1.0,
                                                       cn_half[ln][:], op0=OP.add,
                                                       op1=OP.mult)
                        fc_fb = work_pool.tile([128, 8], f32, tag=f"fc_fb{ln}",
                                               name=f"fc_fb{ln}")
                        nc.vector.scalar_tensor_tensor(fc_fb[:], gt[:, 32:40], 1.0,
                                                       cn_half[ln][:], op0=OP.add,
                                                       op1=OP.mult)
                        nc.vector.scalar_tensor_tensor(stg[("c", ln)][:, sl],
                                                       iz_i[:], 0.5, fc_f[:],
                                                       op0=OP.mult, op1=OP.add)
                        nc.vector.scalar_tensor_tensor(stg[("cb", ln)][:, sl],
                                                       iz_ib[:], 0.5, fc_fb[:],
                                                       op0=OP.mult, op1=OP.add)
                        nc.vector.tensor_scalar(stg[("o", ln)][:, sl], gt[:, 40:48],
                                                1.0, 0.5, op0=OP.add, op1=OP.mult)

                        # --- decay + new state ---
                        dd = work_pool.tile([128, 8], f32, tag=f"dd{ln}",
                                            name=f"dd{ln}")
                        nc.vector.tensor_tensor(dd[:], stg[("c", ln)][:, sl],
                                                stg[("cb", ln)][:, sl],
                                                op=OP.subtract)
                        de = work_pool.tile([128, 8], f32, tag=f"de{ln}",
                                            name=f"de{ln}")
                        nc.vector.tensor_tensor(de[:], dd[:], et[:], op=OP.mult)
                        ctt = work_pool.tile([128, 8], f32, tag=f"ctt{ln}",
                                             name=f"ctt{ln}")
                        nc.vector.tensor_tensor(ctt[:], de[:],
                                                stg[("cb", ln)][:, sl], op=OP.add)
                        tct = work_pool.tile([128, 8], f32, tag=f"tct{ln}",
                                             name=f"tct{ln}")
                        nc.scalar.activation(tct[:], ctt[:], AF.Tanh)
                        # state stays unmasked: outputs are masked at flush,
                        # and post-seq_len state never feeds a valid output.
                        hn_bf[ln] = state_pool.tile([128, 8], bf16,
                                                    tag=f"hn_bf{ln}",
                                                    name=f"hn_bf{ln}")
                        nc.vector.tensor_tensor(hn_bf[ln][:],
                                                stg[("o", ln)][:, sl],
                                                tct[:], op=OP.mult)
                        cn_half[ln] = state_pool.tile([128, 8], f32,
                                                      tag=f"cn_half{ln}",
                                                      name=f"cn_half{ln}")
                        nc.vector.tensor_scalar(cn_half[ln][:], ctt[:], 0.5,
                                                None, op0=OP.mult)

                        if kappa == 7 and 'flush' not in DBG_SKIP:
                            emit_flush(ln, blk)
                    return chain

                def emit_flush(ln, blk):
                    mcol_ap = mcol[:, blk * 2 + ln: blk * 2 + ln + 1]

                    def out_view(oi):
                        return outs[oi][ln * LB:(ln + 1) * LB,
                                        blk * 8 + 1: blk * 8 + 9, :] \
                            .rearrange("b t (c h) -> t c b h", c=2)

                    for oi, nm in ((0, "c"), (1, "cb"), (2, "o"), (3, "d")):
                        tp = tp_pool.tile([128, 128], f32, tag="tp",
                                          name=f"tp_{nm}")
                        nc.tensor.transpose(tp[0:64, :], stg[(nm, ln)][:],
                                            ident[:])
                        om = omask_pool.tile([128, 128], f16, tag=f"om_{nm}",
                                             name=f"om_{nm}")
                        nc.vector.tensor_scalar_mul(om[0:64, :], tp[0:64, :],
                                                    mcol_ap[0:64])
                        nc.sync.dma_start(out_view(oi), om[0:64, :])

                pending = []
                for t in range(L):
                    for ln in range(NLANE):
                        g_d = emit_mms(ln, t)
                        if 'chain' not in DBG_SKIP:
                            if pending:
                                pending.pop(0)()
                            pending.append(make_chain(ln, t, g_d))
                while pending:
                    pending.pop(0)()

            # ---------- Phase 3: ragged pack via indirect gather ----------
            if 'pack' not in DBG_SKIP:
                with tc.tile_pool(name="pk_idx", bufs=1) as pk_idx_pool, \
                     tc.tile_pool(name="pk_stage", bufs=6) as pk_stage_pool:
                    pidx = pk_idx_pool.tile([128, PACK // 128], i32)
                    nc.sync.dma_start(pidx[:], pidx_in[:])
                    for oi in range(4):
                        src = outs[oi][:].rearrange("b t h -> (b t) h")
                        for ch in range(PACK // 128):
                            stage = pk_stage_pool.tile([128, H], f16,
                                                       tag="pkst")
                            nc.gpsimd.indirect_dma_start(
                                out=stage[:], out_offset=None,
                                in_=src,
                                in_offset=bass.IndirectOffsetOnAxis(
                                    ap=pidx[:, ch:ch + 1], axis=0))
                            nc.sync.dma_start(
                                outp[oi * PACK + ch * 128:
                                     oi * PACK + (ch + 1) * 128, :],
                                stage[:])

    nc.finalize()
    # The module never reads partition_id; dropping its allocation saves
    # one per-call binding RPC.  Fall back silently if not removable.
    try:
        import concourse.mybir as _mybir
        f0 = nc.m.functions[0]
        for a in list(f0.allocations):
            if (isinstance(a, _mybir.MemoryLocationSet) and a.memorylocations
                    and a.memorylocations[0].name == "partition_id"):
                f0.allocations.remove(a)
                nc.partition_id_tensor = None
                break
    except Exception:
        pass
    return nc


def _prep_shared(Wx, bx, Wh, bh):
    import ml_dtypes
    Wh_p = Wh[PERM_ROWS].astype(np.float32).copy()
    Wx_p = Wx[PERM_ROWS].astype(np.float32).copy()
    bias_p = (bx + bh)[PERM_ROWS].astype(np.float32).copy()
    for zb in Z_BLOCKS:
        Wh_p[zb * 128:(zb + 1) * 128] *= 2.0
        Wx_p[zb * 128:(zb + 1) * 128] *= 2.0
        bias_p[zb * 128:(zb + 1) * 128] *= 2.0

    win = np.zeros((128, 2 * 28 * 128), dtype=ml_dtypes.bfloat16)
    for j in range(NT):
        for k in range(2):
            s = (2 * j + k) * 128
            win[:, s:s + 128] = Wh_p[j * 128:(j + 1) * 128,
                                     k * 128:(k + 1) * 128].T
            win[:, 28 * 128 + s:28 * 128 + s + 128] = \
                Wx_p[j * 128:(j + 1) * 128, k * 128:(k + 1) * 128].T
    biasg = np.zeros((128, NT), dtype=np.float32)
    for j in range(NT):
        biasg[:, j] = bias_p[j * 128:(j + 1) * 128]
    return win, biasg


def _prep_core(xc, dtc, slc, L, poslen=None):
    import ml_dtypes
    if poslen is None:
        poslen = (L,) * BC
    PACKX = sum(poslen)
    x_rows = xc.reshape(BC * L, I).astype(np.float32)
    xTf = x_rows.T  # [I, BCL]
    xT = np.zeros((128, 2 * PACKX), ml_dtypes.bfloat16)
    off = 0
    for b in range(BC):
        n = poslen[b]
        xT[:, off:off + n] = xTf[:128, b * L:b * L + n]
        xT[:, PACKX + off:PACKX + off + n] = xTf[128:, b * L:b * L + n]
        off += n
    t_idx = np.arange(L)
    m = (t_idx[None, :] < slc[:, None]).astype(np.float32)  # [BC, L]
    dt2 = dtc[:, :, 0].astype(np.float32)  # [BC, L]
    # [1, L*16]: column t*16 + lane*8 + c*4 + b' -> value for (b, t)
    # where b = lane*4 + b'
    col_dt = np.empty((L, 2, 2, LB), np.float32)
    for ln in range(NLANE):
        for c in range(2):
            col_dt[:, ln, c, :] = dt2[ln * LB:(ln + 1) * LB, :].T
    dtrow = col_dt.reshape(L * 16)
    # mcolT [128, 2*NBLK]: partition p = kappa*8 + c*4 + b', col = blk*2+lane
    NBLK = L // 8
    mcol = np.zeros((128, 2 * NBLK), dtype=np.float32)
    for blk in range(NBLK):
        for ln in range(NLANE):
            v = m[ln * LB:(ln + 1) * LB, blk * 8:blk * 8 + 8]  # [b', kappa]
            col = np.repeat(v.T[:, None, :], 2, axis=1)  # [kappa, c, b']
            mcol[0:64, blk * 2 + ln] = col.reshape(64)
    return xT, dtrow, mcol


class _CachedRunner:
    """Build the sharded jitted executable once; reuse across calls so the
    NEFF is loaded on the devices a single time.  Output zero-buffers are
    created on-device (never uploaded)."""

    def __init__(self, nc):
        sys.path.insert(0, "/opt/trn_rl_repo")
        import jax
        import jax.numpy as jnp
        import numpy as _np
        from jax.sharding import Mesh, PartitionSpec, NamedSharding
        from jax.experimental.shard_map import shard_map
        from concourse import mybir
        from concourse.bass2jax import _bass_exec_p, partition_id_tensor, \
            install_neuronx_cc_hook
        install_neuronx_cc_hook()
        self.jax = jax
        partition_name = (nc.partition_id_tensor.name
                          if nc.partition_id_tensor else None)
        in_names, out_names, out_avals = [], [], []
        for alloc in nc.m.functions[0].allocations:
            if not isinstance(alloc, mybir.MemoryLocationSet):
                continue
            name = alloc.memorylocations[0].name
            if alloc.kind == "ExternalInput":
                if name != partition_name:
                    in_names.append(name)
            elif alloc.kind == "ExternalOutput":
                out_names.append(name)
                shape = tuple(alloc.tensor_shape)
                dtype = mybir.dt.np(alloc.dtype)
                out_avals.append(jax.core.ShapedArray(shape, dtype))
        self.n_params = len(in_names)
        self.in_names = list(in_names)
        self.out_names = out_names
        self.out_avals = out_avals
        in_names_all = in_names + out_names
        if partition_name is not None:
            in_names_all.append(partition_name)

        def _body(*args):
            operands = list(args)
            if partition_name is not None:
                operands.append(partition_id_tensor())
            outs = _bass_exec_p.bind(
                *operands, out_avals=tuple(out_avals),
                in_names=tuple(in_names_all), out_names=tuple(out_names),
                lowering_input_output_aliases=(), sim_require_finite=True,
                sim_require_nnan=True, nc=nc)
            return tuple(outs)

        n_outs = len(out_avals)
        devices = jax.devices()[:NCORES]
        mesh = Mesh(_np.asarray(devices), ("core",))
        sharding = NamedSharding(mesh, PartitionSpec("core"))
        self._sharding = sharding
        in_specs = (PartitionSpec("core"),) * (self.n_params + n_outs)
        out_specs = (PartitionSpec("core"),) * n_outs
        donate = tuple(range(self.n_params, self.n_params + n_outs))
        self.sharded = jax.jit(
            shard_map(_body, mesh=mesh, in_specs=in_specs,
                      out_specs=out_specs, check_rep=False),
            donate_argnums=donate, keep_unused=True)
        # zero output buffers are created on-device (never uploaded)
        self._zeros_fn = jax.jit(
            lambda: tuple(
                jnp.zeros((NCORES * a.shape[0], *a.shape[1:]), a.dtype)
                for a in out_avals),
            out_shardings=(sharding,) * n_outs)

    # inputs that usually don't change between calls -- keep them
    # device-resident across calls keyed by content digest.
    _STABLE = {"win", "fin", "pidx"}

    def __call__(self, in_maps):
        import numpy as _np
        import hashlib
        if not hasattr(self, "_stable_cache"):
            self._stable_cache = {}
            self._prev_out = None
        concat_in = []
        for i, name in enumerate(self.in_names):
            arrs = [_np.asarray(m[name]) for m in in_maps]
            cat = _np.concatenate(arrs, axis=0)
            if name in self._STABLE:
                dig = hashlib.blake2b(cat.tobytes(), digest_size=16).digest()
                hit = self._stable_cache.get(name)
                if hit is not None and hit[0] == dig:
                    concat_in.append(hit[1])
                    continue
                dev = self.jax.device_put(cat, self._sharding)
                self._stable_cache[name] = (dig, dev)
                concat_in.append(dev)
            else:
                concat_in.append(cat)
        # recycle last call's output buffers as this call's donated
        # "zero" buffers -- their initial content is never observed.
        zeros = self._prev_out if self._prev_out is not None \
            else self._zeros_fn()
        out_arrs = self.sharded(*concat_in, *zeros)
        self._prev_out = tuple(out_arrs)
        # return per-core shards unfetched so the caller can overlap
        # host post-processing with the device->host transfer
        out = out_arrs[0]
        rows = self.out_avals[0].shape[0]
        shards = [None] * NCORES
        for s in out.addressable_shards:
            shards[s.index[0].start // rows] = s.data
        return shards


_RUNNER_CACHE = {}


def kernel(x, delta_t, seq_lens, Wx, bx, Wh, bh, _L=None):
    L = _L if _L is not None else x.shape[1]
    x = np.asarray(x)
    delta_t = np.asarray(delta_t)
    seq_lens = np.asarray(seq_lens)
    lens0 = tuple(int(v) for v in seq_lens)
    perm = _balance(lens0)  # perm[c*BC+i] = original batch index
    lens = tuple(lens0[p] for p in perm)
    PACK = _pack_rows(lens, L)
    poslen = tuple(max(lens[k * BC + p] for k in range(NCORES))
                   for p in range(BC))
    key = (L, PACK, poslen)
    if key not in _BUILD_CACHE:
        _BUILD_CACHE[key] = _build(L, pack=PACK, poslen=poslen)
    nc = _BUILD_CACHE[key]
    rkey = id(nc)
    if rkey not in _RUNNER_CACHE:
        _RUNNER_CACHE[rkey] = _CachedRunner(nc)
    runner = _RUNNER_CACHE[rkey]

    win, biasg = _prep_shared(np.asarray(Wx), np.asarray(bx),
                              np.asarray(Wh), np.asarray(bh))
    ident = np.eye(128, dtype=np.float32)

    in_maps = []
    for k in range(NCORES):
        sel = perm[k * BC:(k + 1) * BC]
        xT, dtrow, mcol = _prep_core(x[sel], delta_t[sel], seq_lens[sel], L,
                                     poslen)
        fin = np.concatenate([biasg.ravel(), mcol.ravel(), ident.ravel(),
                              dtrow])[None, :].astype(np.float32)
        # packed-row -> padded-flat-row index table, [128, PACK//128]
        idx = np.zeros(PACK, np.int32)
        pos = 0
        for b in range(BC):
            n = lens[k * BC + b] + 1
            idx[pos:pos + n] = b * (L + 1) + np.arange(n)
            pos += n
        pidx = np.ascontiguousarray(idx.reshape(PACK // 128, 128).T)
        in_maps.append({"win": win, "xin": xT, "fin": fin, "pidx": pidx})

    shards = runner(in_maps)  # per-core device shards [4*PACK, H] f16

    # host-side: reconstruct befores (hn) + afters_h on the packed rows,
    # then scatter all six outputs into full-size zero arrays.  Each
    # thread fetches its core's shard, overlapping transfer and math.
    from concurrent.futures import ThreadPoolExecutor
    full = [np.zeros((B, L + 1, H), np.float32) for _ in range(6)]

    def _post_core(k):
        allout = np.asarray(shards[k])  # [4*PACK, H]
        rows = sum(v + 1 for v in lens[k * BC:(k + 1) * BC])
        c = allout[0 * PACK:0 * PACK + rows].astype(np.float32)
        cb = allout[1 * PACK:1 * PACK + rows].astype(np.float32)
        o = allout[2 * PACK:2 * PACK + rows].astype(np.float32)
        d = allout[3 * PACK:3 * PACK + rows].astype(np.float32)
        dtp = np.zeros((rows, 1), np.float32)
        pos = 0
        for b in range(BC):
            n = lens[k * BC + b] + 1
            dtp[pos + 1:pos + n, 0] = delta_t[perm[k * BC + b], 0:n - 1, 0]
            pos += n
        ah = o * np.tanh(c)
        ct = cb + (c - cb) * np.exp(-d * dtp)
        bef = o * np.tanh(ct)
        arrs = (bef, ah, c, cb, o, d)
        pos = 0
        for b in range(BC):
            n = lens[k * BC + b] + 1
            gb = perm[k * BC + b]
            for i in range(6):
                full[i][gb, 0:n] = arrs[i][pos:pos + n]
            pos += n

    with ThreadPoolExecutor(NCORES) as ex:
        list(ex.map(_post_core, range(NCORES)))
    return tuple(full)
